# revision 10
# baseline (speedup 1.0000x reference)
# MultiHeadAttention forward, sharded over 8 NeuronCores.
#
# Problem: N=4, S=T=2048, E=1024, H=16 heads (DH=64), fp32 in/out.
# Sharding: core = n*2 + hg  (n = batch 0..3, hg = head-group 0..1, 8 heads each).
# Each core computes q/k/v projections for its head group, attention, and a
# partial output projection; the host sums the two partials per batch and adds bp.
#
# Device layout (all contractions need the reduced dim on partitions):
#   - host passes query/key/value TRANSPOSED ([E, S]) and the weights
#     pre-transposed, so every matmul operand loads naturally.
#   - qT, kT computed transposed [512, S] (head-dim on partitions); v natural
#     [T, 512]; scores computed transposed [T, S] so softmax normalization can
#     be deferred: exp() is applied PSUM->SBUF on the Scalar engine (scale=1/8
#     fused), y = v_aug.T @ expT accumulates over T with a ones-column in v_aug
#     producing the softmax denominators for free in psum row 64.
#   - K=64 score matmuls are row-packed in pairs of heads (partitions 0-63 /
#     64-127) for PE subarray concurrency.
#
# Schedule (this file's main difference from the straightforward version):
#   - the Scalar engine (exp) is the long pole (~293us of ACTIVATE); the whole
#     kernel is organized to start it ASAP and never let it stall.
#   - DMAs are emitted in deadline order, T-sliced: the 4MB that gates the
#     first exp (wk + kT cols 0:512 + wq + qT cols 0:512) goes first, then v,
#     then everything else.  First exp fires ~25us in instead of ~82us.
#   - attention over all 17 (pair, S-chunk) calls is ONE software-pipelined
#     stream: scores(i) / exp(i) / y(i-1) where i is a global group slot; the
#     y+normalization of a call overlaps the next call's first scores/exp.
#   - all other PE work (v/k/q projections, out-projection) is placed into
#     the stream via a static slot->tasks map, ordered by deadline: v tiles
#     are produced one group ahead of the y that consumes them, k/q units one
#     call ahead, chunk c's out-projection spread across chunk c+1.
#   - the final (pair 3, chunk 3) call is split into two 256-wide half calls
#     so half A's out-projection overlaps half B's attention; two units are
#     held back to bridge the last normalization chain (keeps HAM warm).
#   - softmax denominators: psum row 64 -> partition 0 -> gpsimd
#     partition_broadcast (raw) -> DVE fast-approx reciprocal; head b's
#     normalize writes yt partitions 64:127 directly (cross-base DVE).
#   - outputs staged/stored as bf16 (partials summed on host in fp32).

import numpy as np
import ml_dtypes
from collections import defaultdict

N, S, T, E, H = 4, 2048, 2048, 1024, 16
DH = E // H  # 64
HC = H // 2  # heads per core = 8
EC = HC * DH  # per-core head width = 512
SCALE = float(np.sqrt(DH))  # 8.0
P = 128
KT = E // P  # 8 contraction tiles for projections
NT = T // P  # 16 T-tiles
NC_ = 8  # cores

# matmul/IO dtype on device
_DT_NAME = "bfloat16"

_cache = {}


def _build_nc(debug_dumps=False, reps=1):
    import concourse.bass as bass
    import concourse.mybir as mybir
    import concourse.tile as tile
    from concourse import bacc

    DT = getattr(mybir.dt, _DT_NAME)
    F32 = mybir.dt.float32
    AF = mybir.ActivationFunctionType
    ALU = mybir.AluOpType

    nc = bacc.Bacc("TRN2", target_bir_lowering=False, debug=False, num_devices=NC_)

    qT_in = nc.dram_tensor("qT_in", [E, S], DT, kind="ExternalInput")
    kT_in = nc.dram_tensor("kT_in", [E, T], DT, kind="ExternalInput")
    vT_in = nc.dram_tensor("vT_in", [E, T], DT, kind="ExternalInput")
    wqT = nc.dram_tensor("wqT", [E, EC], DT, kind="ExternalInput")
    wkT = nc.dram_tensor("wkT", [E, EC], DT, kind="ExternalInput")
    wvT = nc.dram_tensor("wvT", [E, EC], DT, kind="ExternalInput")
    wpT = nc.dram_tensor("wpT", [EC, E], DT, kind="ExternalInput")
    bq_d = nc.dram_tensor("bq", [EC], F32, kind="ExternalInput")
    bk_d = nc.dram_tensor("bk", [EC], F32, kind="ExternalInput")
    bv_d = nc.dram_tensor("bv", [EC], F32, kind="ExternalInput")
    out_d = nc.dram_tensor("out", [S, E], DT, kind="ExternalOutput")

    SCH = 512  # S-chunk width (scores rhs free size)
    NCH = S // SCH  # 4 chunks
    TG = 2  # T-tiles per exp group
    NG = NT // TG  # 8 groups per call
    HW = SCH // 2  # width of the two tail half-calls

    inv_scale = 1.0 / SCALE

    with tile.TileContext(nc) as tc:
        with (
            tc.tile_pool(name="const", bufs=1) as const,
            tc.tile_pool(name="wbuf", bufs=1) as wbuf,
            tc.tile_pool(name="kinp", bufs=1) as kinpool,
            tc.tile_pool(name="qin0p", bufs=1) as qin0pool,
            tc.tile_pool(name="bigin", bufs=1) as bigin,
            tc.tile_pool(name="wp", bufs=1) as wppool,
            tc.tile_pool(name="qt", bufs=1) as qtpool,
            tc.tile_pool(name="kt", bufs=1) as ktpool,
            tc.tile_pool(name="vsb", bufs=1) as vpool,
            tc.tile_pool(name="yt", bufs=1) as ytpool,
            tc.tile_pool(name="expb", bufs=8) as expool,
            tc.tile_pool(name="norm", bufs=1) as norm,
            tc.tile_pool(name="ostage", bufs=2) as ostage,
            tc.tile_pool(name="ps_sc", bufs=2, space="PSUM") as ps_sc,
            tc.tile_pool(name="ps_f", bufs=2, space="PSUM") as ps_f,
            tc.tile_pool(name="ps_y", bufs=1, space="PSUM") as ps_y,
        ):
            for _rep in range(reps):
                # ---- exp table preload + PE warm-up burst: get the ~2.7us
                # ACT table-set load and the ~3.4us HAM clock-warm window out
                # of the way while the first DMAs are still in flight ----
                scr = const.tile([1, 16], F32, tag="scr")
                nc.gpsimd.memset(scr[:, :], 0.0)
                nc.scalar.activation(scr[0:1, 8:16], scr[0:1, 0:8], AF.Exp)
                scrw = const.tile([P, SCH], DT, tag="scrw")
                nc.gpsimd.memset(scrw[:, :], 0.0)
                warm_ps = ps_sc.tile([P, TG * SCH], F32, tag="ps", name="warm_ps")

                def warm(n):
                    for _ in range(n):
                        nc.tensor.matmul(
                            warm_ps[:, 0:SCH],
                            lhsT=scrw[:, 0:P],
                            rhs=scrw[:, :],
                            start=True,
                            stop=True,
                        )

                warm(6)

                # ---- constants / biases ----
                bq_sb = const.tile([P, KT // 2], F32, tag="bq")  # [128, 4]
                bk_sb = const.tile([P, KT // 2], F32, tag="bk")
                bvrow = const.tile([P, EC], F32, tag="bvrow")
                bvrep = const.tile([P, EC], F32, tag="bvrep")

                # persistent activation tensors
                qt_t = [
                    qtpool.tile([P, S], DT, tag=f"qt{j}", name=f"qt{j}")
                    for j in range(4)
                ]
                kt_t = [
                    ktpool.tile([P, T], DT, tag=f"kt{j}", name=f"kt{j}")
                    for j in range(4)
                ]
                v_t = [
                    vpool.tile([P, HC * (DH + 1)], DT, tag=f"v{t}", name=f"v{t}")
                    for t in range(NT)
                ]
                yt_t = [
                    ytpool.tile([P, S], DT, tag=f"yt{j}", name=f"yt{j}")
                    for j in range(4)
                ]

                # ones columns of v_aug (col 64 of each 65-wide head block)
                for t in range(NT):
                    vv = v_t[t].rearrange("p (h d) -> p h d", h=HC)
                    nc.gpsimd.memset(vv[:, :, DH : DH + 1], 1.0)

                # ---- inputs/weights: ONE batched 3D DMA per logical block
                # (a dma_start costs ~0.6us of serial Sync-engine issue time,
                # so the exp-gating loads must be few and fat), emitted in
                # deadline order: wk + kT chunk0 + wq + qT chunk0 gate the
                # first exp; v next; everything else streams behind. ----
                # layouts: w*_all[p, (k o)]; kin/vin[p, (cc k t)]
                wk_all = wbuf.tile([P, KT * EC], DT, tag="wk", name="wk_all")
                wq_all = wbuf.tile([P, KT * EC], DT, tag="wq", name="wq_all")
                wv_all = wbuf.tile([P, KT * EC], DT, tag="wv", name="wv_all")
                kin = kinpool.tile([P, KT * T], DT, tag="kin", name="kin_all")
                qin0 = qin0pool.tile([P, KT * SCH], DT, tag="qin0", name="qin0_all")
                vin = bigin.tile([P, KT * T], DT, tag="big", name="vin_all")
                CW = KT * SCH  # columns per (cc) chunk in kin/vin layouts

                def dma_in_chunk(dst_all, src, cc, k0=0, k1=KT):
                    nc.sync.dma_start(
                        dst_all[
                            :, cc * CW + k0 * SCH : cc * CW + k1 * SCH
                        ].rearrange("p (k t) -> p k t", k=k1 - k0),
                        src[k0 * P : k1 * P, cc * SCH : (cc + 1) * SCH].rearrange(
                            "(k p) t -> p k t", p=P
                        ),
                    )

                def dma_w(dst_all, src, k0=0, k1=KT):
                    nc.sync.dma_start(
                        dst_all[:, k0 * EC : k1 * EC].rearrange(
                            "p (k o) -> p k o", k=k1 - k0
                        ),
                        src[k0 * P : k1 * P, :].rearrange("(k p) o -> p k o", p=P),
                    )

                def dma_q0(k0, k1):
                    nc.sync.dma_start(
                        qin0[:, k0 * SCH : k1 * SCH].rearrange(
                            "p (k t) -> p k t", k=k1 - k0
                        ),
                        qT_in[k0 * P : k1 * P, 0:SCH].rearrange(
                            "(k p) t -> p k t", p=P
                        ),
                    )

                # exp-gating loads split in halves so the k/q projection
                # matmuls pipeline behind the DMA instead of waiting for
                # whole-block completion
                dma_w(wk_all, wkT, 0, 4)
                dma_in_chunk(kin, kT_in, 0, 0, 4)
                dma_w(wk_all, wkT, 4, KT)
                dma_in_chunk(kin, kT_in, 0, 4, KT)
                dma_w(wq_all, wqT, 0, 4)
                dma_q0(0, 4)
                dma_w(wq_all, wqT, 4, KT)
                dma_q0(4, KT)
                nc.sync.dma_start(bq_sb[:, :], bq_d.rearrange("(m p) -> p m", p=P))
                nc.sync.dma_start(bk_sb[:, :], bk_d.rearrange("(m p) -> p m", p=P))
                nc.sync.dma_start(bvrow[0:1, :], bv_d[None, :])
                nc.gpsimd.partition_broadcast(bvrep[:, :], bvrow[0:1, :])
                dma_in_chunk(kin, kT_in, 1)
                dma_w(wv_all, wvT)
                dma_in_chunk(vin, vT_in, 0)
                dma_in_chunk(vin, vT_in, 1)
                dma_in_chunk(kin, kT_in, 2)
                dma_in_chunk(kin, kT_in, 3)
                dma_in_chunk(vin, vT_in, 2)
                dma_in_chunk(vin, vT_in, 3)
                # qT chunks 1-3 reuse vin's slot (WAR on the last v-proj read)
                qin_r = bigin.tile([P, KT * (S - SCH)], DT, tag="big", name="qinr")
                for cc in range(3):
                    nc.sync.dma_start(
                        qin_r[:, cc * CW : (cc + 1) * CW].rearrange(
                            "p (k t) -> p k t", k=KT
                        ),
                        qT_in[:, (cc + 1) * SCH : (cc + 2) * SCH].rearrange(
                            "(k p) t -> p k t", p=P
                        ),
                    )
                wp_sb = wppool.tile([P, 4 * E], DT, tag="wp")
                nc.sync.dma_start(
                    wp_sb.rearrange("p (k o) -> p k o", k=4),
                    wpT.rearrange("(k p) o -> p k o", p=P),
                )

                # ---- unit builders (psum from the dedicated 1-bank filler
                # pool so the scores double-buffer rotation stays stable) ----
                def proj_unit(j, w_all, act_slice, bias_sb, dst_slice, split):
                    """act_slice(k) -> rhs AP; returns 1 or 2 emit closures."""
                    state = {}

                    def emit_half(k0, k1, first, last):
                        def emit():
                            if first:
                                state["ps"] = ps_f.tile(
                                    [P, SCH], F32, tag="fps", name="fps"
                                )
                            ps = state["ps"]
                            for k in range(k0, k1):
                                nc.tensor.matmul(
                                    ps[:, :],
                                    lhsT=w_all[:, k * EC + j * P : k * EC + (j + 1) * P],
                                    rhs=act_slice(k),
                                    start=(k == 0),
                                    stop=(k == KT - 1),
                                )
                            if last:
                                nc.vector.tensor_scalar(
                                    dst_slice,
                                    ps[:, :],
                                    bias_sb[:, j : j + 1],
                                    None,
                                    ALU.add,
                                )

                        return emit

                    if split:
                        return [emit_half(0, 4, True, False),
                                emit_half(4, KT, False, True)]
                    return [emit_half(0, KT, True, True)]

                def k_unit(j, cc, split=False):
                    return proj_unit(
                        j, wk_all,
                        lambda k: kin[:, cc * CW + k * SCH : cc * CW + (k + 1) * SCH],
                        bk_sb, kt_t[j][:, cc * SCH : (cc + 1) * SCH], split,
                    )

                def q_unit(j, cc, split=False):
                    if cc == 0:
                        act = lambda k: qin0[:, k * SCH : (k + 1) * SCH]
                    else:
                        act = lambda k: qin_r[
                            :, (cc - 1) * CW + k * SCH : (cc - 1) * CW + (k + 1) * SCH
                        ]
                    return proj_unit(
                        j, wq_all, act, bq_sb,
                        qt_t[j][:, cc * SCH : (cc + 1) * SCH], split,
                    )

                def v_unit(t):
                    def emit():
                        ps = ps_f.tile([P, SCH], F32, tag="fps", name="fps")
                        cc, tl = t // 4, t % 4
                        for k in range(KT):
                            nc.tensor.matmul(
                                ps[:, 0:EC],
                                lhsT=vin[
                                    :,
                                    cc * CW + k * SCH + tl * P :
                                    cc * CW + k * SCH + (tl + 1) * P,
                                ],
                                rhs=wv_all[:, k * EC : (k + 1) * EC],
                                start=(k == 0),
                                stop=(k == KT - 1),
                            )
                        vv = v_t[t].rearrange("p (h d) -> p h d", h=HC)
                        nc.vector.tensor_tensor(
                            vv[:, :, 0:DH],
                            ps[:, 0:EC].rearrange("p (h d) -> p h d", h=HC),
                            bvrep.rearrange("p (h d) -> p h d", h=HC),
                            ALU.add,
                        )

                    return emit

                def out_proj_unit(c, mi, n):
                    def emit():
                        ps = ps_f.tile([P, SCH], F32, tag="fps", name="fps")
                        for kp in range(4):
                            nc.tensor.matmul(
                                ps[:, 0:EC],
                                lhsT=yt_t[kp][
                                    :, c * SCH + mi * P : c * SCH + (mi + 1) * P
                                ],
                                rhs=wp_sb[
                                    :, kp * E + n * EC : kp * E + (n + 1) * EC
                                ],
                                start=(kp == 0),
                                stop=(kp == 3),
                            )
                        ost = ostage.tile([P, EC], DT, tag="ost")
                        nc.vector.tensor_copy(ost[:, :], ps[:, 0:EC])
                        nc.sync.dma_start(
                            out_d[
                                c * SCH + mi * P : c * SCH + (mi + 1) * P,
                                n * EC : (n + 1) * EC,
                            ],
                            ost[:, :],
                        )

                    return emit

                # ---- attention stream pieces ----
                def emit_scores_exp(st, g):
                    j, w, soff = st["j"], st["w"], st["soff"]
                    ps_a = ps_sc.tile([P, TG * SCH], F32, tag="ps", name="ps_a")
                    ps_b = ps_sc.tile([P, TG * SCH], F32, tag="ps", name="ps_b")
                    for i in range(TG):
                        t = g * TG + i
                        nc.tensor.matmul(
                            ps_a[:, i * w : (i + 1) * w],
                            lhsT=kt_t[j][0:DH, t * P : (t + 1) * P],
                            rhs=qt_t[j][0:DH, soff : soff + w],
                            start=True,
                            stop=True,
                        )
                        nc.tensor.matmul(
                            ps_b[:, i * w : (i + 1) * w],
                            lhsT=kt_t[j][DH:P, t * P : (t + 1) * P],
                            rhs=qt_t[j][DH:P, soff : soff + w],
                            start=True,
                            stop=True,
                        )
                    exp_a = expool.tile([P, TG * SCH], DT, tag="exp", name="exp_a")
                    exp_b = expool.tile([P, TG * SCH], DT, tag="exp", name="exp_b")
                    nc.scalar.activation(
                        exp_a[:, 0 : TG * w], ps_a[:, 0 : TG * w], AF.Exp,
                        scale=inv_scale,
                    )
                    nc.scalar.activation(
                        exp_b[:, 0 : TG * w], ps_b[:, 0 : TG * w], AF.Exp,
                        scale=inv_scale,
                    )
                    st["exps"][g] = (exp_a, exp_b)

                def emit_y(st, g):
                    # head b always accumulates at column offset SCH (its own
                    # PSUM bank): matmul start=True clears has_written bits
                    # bank-wide, so the two heads' accumulation groups must
                    # not share a bank even for the half-width tail calls.
                    j, w = st["j"], st["w"]
                    if st["ps_yab"] is None:
                        st["ps_yab"] = ps_y.tile(
                            [P, 2 * SCH], F32, tag="ps_yab", name="ps_yab"
                        )
                    ps_yab = st["ps_yab"]
                    exp_a, exp_b = st["exps"].pop(g)
                    for i in range(TG):
                        t = g * TG + i
                        va = v_t[t][:, (2 * j) * (DH + 1) : (2 * j + 1) * (DH + 1)]
                        vb = v_t[t][
                            :, (2 * j + 1) * (DH + 1) : (2 * j + 2) * (DH + 1)
                        ]
                        nc.tensor.matmul(
                            ps_yab[0 : DH + 1, 0:w],
                            lhsT=va,
                            rhs=exp_a[:, i * w : (i + 1) * w],
                            start=(t == 0),
                            stop=(t == NT - 1),
                        )
                        nc.tensor.matmul(
                            ps_yab[0 : DH + 1, SCH : SCH + w],
                            lhsT=vb,
                            rhs=exp_b[:, i * w : (i + 1) * w],
                            start=(t == 0),
                            stop=(t == NT - 1),
                        )

                def emit_norm(st):
                    # unnormalized y (incl. denominator row 64) copied psum ->
                    # SBUF per head (shortest WAR on the y psum banks for the
                    # next call); raw denominator rows partition-broadcast on
                    # the idle GpSimd engine, then fast-approx DVE reciprocal
                    # (~18 bits, plenty for softmax denoms; base partition
                    # must be 0 for the custom op).  No DMA hops anywhere.
                    j, w, soff = st["j"], st["w"], st["soff"]
                    ps_yab = st["ps_yab"]
                    yun = norm.tile([P, 2 * SCH], F32, tag="yun")
                    den0 = norm.tile([1, 2 * SCH], F32, tag="den0")
                    nc.vector.tensor_copy(
                        den0[0:1, 0:w], ps_yab[DH : DH + 1, 0:w]
                    )
                    nc.vector.tensor_copy(
                        den0[0:1, w : 2 * w], ps_yab[DH : DH + 1, SCH : SCH + w]
                    )
                    nc.vector.tensor_copy(yun[0:DH, 0:w], ps_yab[0:DH, 0:w])
                    nc.vector.tensor_copy(
                        yun[0:DH, w : 2 * w], ps_yab[0:DH, SCH : SCH + w]
                    )
                    recraw = norm.tile([P, 2 * SCH], F32, tag="recraw")
                    nc.gpsimd.partition_broadcast(
                        recraw[0:DH, 0 : 2 * w], den0[0:1, 0 : 2 * w]
                    )
                    recrep = norm.tile([P, 2 * SCH], F32, tag="recrep")
                    nc.vector.reciprocal_approx_fast(
                        recrep[0:DH, 0 : 2 * w], recraw[0:DH, 0 : 2 * w]
                    )
                    nc.vector.tensor_tensor(
                        yt_t[j][0:DH, soff : soff + w],
                        yun[0:DH, 0:w],
                        recrep[0:DH, 0:w],
                        ALU.mult,
                    )
                    # head b writes its yt half directly at partitions 64:128
                    nc.vector.tensor_tensor(
                        yt_t[j][DH:P, soff : soff + w],
                        yun[0:DH, w : 2 * w],
                        recrep[0:DH, w : 2 * w],
                        ALU.mult,
                    )

                # ---- call list: 15 full calls + 2 half calls for the tail ----
                calls = [
                    {"c": c, "j": j, "soff": c * SCH, "w": SCH,
                     "exps": {}, "ps_yab": None}
                    for c in range(NCH)
                    for j in range(4)
                ]

                # ---- static slot -> filler tasks (deadline-ordered).
                # k/q units appended first within a slot (they gate scores ->
                # ACT), v units after (their consumer, y, has slack). ----
                slot_tasks = defaultdict(list)
                for cc in range(1, 4):  # k(0,0) is emitted pre-stream
                    slot_tasks[2 * cc - 1] += k_unit(0, cc)
                for j in range(1, 4):
                    for cc in range(4):
                        slot_tasks[8 * j + 2 * cc - 2] += k_unit(j, cc)
                    slot_tasks[8 * j - 2] += q_unit(j, 0)
                for c in range(1, 4):
                    for j in range(4):
                        h1, h2 = q_unit(j, c, split=True)
                        slot_tasks[32 * c + 8 * j - 7].append(h1)
                        slot_tasks[32 * c + 8 * j - 6].append(h2)
                for t in range(NT):
                    slot_tasks[t // 2 + 1].append(v_unit(t))
                # out-projection: chunk c spread over chunk c+1; chunk 2's
                # over the chunk-3 calls; chunk 3's handled in the tail.
                for c in range(2):
                    for i, (mi, n) in enumerate(
                        (mi, n) for mi in range(4) for n in range(2)
                    ):
                        slot_tasks[32 * (c + 1) + 2 + i].append(
                            out_proj_unit(c, mi, n)
                        )
                c2_units = [out_proj_unit(2, mi, n) for mi in range(4) for n in range(2)]
                for i, fn in enumerate(c2_units[:5]):
                    slot_tasks[98 + i].append(fn)
                # three chunk-2 units are held back to bridge the final
                # normalization chain (keeps the PE HAM-warm into the tail)
                bridge_units = c2_units[5:]

                # chunk 3's units stage into a reused SBUF block (qin0's slot
                # is long dead) and ship as ONE batched DMA: a dma_start
                # costs ~0.7us of serial Sync issue, x8 it would dominate the
                # tail.
                tailstage = qin0pool.tile(
                    [P, KT * SCH], DT, tag="qin0", name="tailstage"
                )

                def tail_unit(mi, n):
                    def emit():
                        ps = ps_f.tile([P, SCH], F32, tag="fps", name="fps")
                        for kp in range(4):
                            nc.tensor.matmul(
                                ps[:, 0:EC],
                                lhsT=yt_t[kp][
                                    :, 3 * SCH + mi * P : 3 * SCH + (mi + 1) * P
                                ],
                                rhs=wp_sb[
                                    :, kp * E + n * EC : kp * E + (n + 1) * EC
                                ],
                                start=(kp == 0),
                                stop=(kp == 3),
                            )
                        nc.vector.tensor_copy(
                            tailstage[:, (mi * 2 + n) * EC : (mi * 2 + n + 1) * EC],
                            ps[:, 0:EC],
                        )

                    return emit

                tail_units = [tail_unit(mi, n) for mi in range(4) for n in range(2)]

                # ---- bootstrap: what the first scores need, padded with
                # dependency-free warm-up matmuls so the PE never idles long
                # enough during the DMA-paced ramp for HAM to re-throttle ----
                k00h1, k00h2 = k_unit(0, 0, split=True)
                q00h1, q00h2 = q_unit(0, 0, split=True)
                k00h1()
                warm(4)
                k00h2()
                warm(4)
                q00h1()
                warm(3)
                q00h2()
                warm(2)

                # ---- the software-pipelined stream (y lags 2 slots so a new
                # call's first y never head-of-line blocks the PE queue on
                # the previous call's normalization psum reads) ----
                pending = []
                for ci, st in enumerate(calls):
                    for g in range(NG):
                        slot = 8 * ci + g
                        emit_scores_exp(st, g)
                        if len(pending) == 2:
                            pst, pg = pending.pop(0)
                            emit_y(pst, pg)
                            if pg == NG - 1:
                                emit_norm(pst)
                        pending.append((st, g))
                        for fn in slot_tasks.get(slot, ()):
                            fn()
                # drain: final two y's + normalization, HAM-bridged by the
                # reserved chunk-2 units running during the norm chain
                for pst, pg in pending:
                    emit_y(pst, pg)
                    if pg == NG - 1:
                        emit_norm(pst)
                for fn in bridge_units:
                    fn()
                for fn in tail_units:
                    fn()
                nc.sync.dma_start(
                    out_d[3 * SCH : 4 * SCH, :].rearrange(
                        "(mi p) e -> p mi e", p=P
                    ),
                    tailstage.rearrange("p (mi e) -> p mi e", mi=4),
                )

    nc.compile()
    return nc


def _get_nc():
    if "nc" not in _cache:
        _cache["nc"] = _build_nc()
    return _cache["nc"]


def _make_runner(nc):
    """Build a reusable sharded jit callable (mirrors bass2jax.run_bass_via_pjrt)."""
    import jax
    import numpy as np
    import concourse.mybir as mybir
    from concourse import bass2jax
    from concourse.bass2jax import _bass_exec_p, partition_id_tensor
    from jax.sharding import Mesh, PartitionSpec, NamedSharding
    from jax.experimental.shard_map import shard_map

    bass2jax.install_neuronx_cc_hook()

    partition_name = nc.partition_id_tensor.name if nc.partition_id_tensor else None
    in_names, out_names, out_avals, zero_shapes = [], [], [], []
    for alloc in nc.m.functions[0].allocations:
        if not isinstance(alloc, mybir.MemoryLocationSet):
            continue
        name = alloc.memorylocations[0].name
        if alloc.kind == "ExternalInput":
            if name != partition_name:
                in_names.append(name)
        elif alloc.kind == "ExternalOutput":
            out_names.append(name)
            shape = tuple(alloc.tensor_shape)
            dtype = mybir.dt.np(alloc.dtype)
            out_avals.append(jax.core.ShapedArray(shape, dtype))
            zero_shapes.append((shape, dtype))
    n_params = len(in_names)
    n_outs = len(out_avals)
    all_names = list(in_names) + list(out_names)
    if partition_name is not None:
        all_names.append(partition_name)

    def _body(*args):
        operands = list(args)
        if partition_name is not None:
            operands.append(partition_id_tensor())
        outs = _bass_exec_p.bind(
            *operands,
            out_avals=tuple(out_avals),
            in_names=tuple(all_names),
            out_names=tuple(out_names),
            lowering_input_output_aliases=(),
            sim_require_finite=True,
            sim_require_nnan=True,
            nc=nc,
        )
        return tuple(outs)

    devices = jax.devices()[:NC_]
    mesh = Mesh(np.asarray(devices), ("core",))
    in_specs = (PartitionSpec("core"),) * (n_params + n_outs)
    out_specs = (PartitionSpec("core"),) * n_outs
    sharded = jax.jit(
        shard_map(
            _body, mesh=mesh, in_specs=in_specs, out_specs=out_specs, check_rep=False
        ),
        keep_unused=True,
    )

    shd = NamedSharding(mesh, PartitionSpec("core"))

    def place(in_maps):
        """device_put the concatenated inputs + zero out-buffers once."""
        per_core = [[np.asarray(m[name]) for name in in_names] for m in in_maps]
        concat_in = [
            np.concatenate([per_core[c][i] for c in range(NC_)], axis=0)
            for i in range(n_params)
        ]
        concat_zeros = [
            np.zeros((NC_ * sh[0], *sh[1:]), dt) for sh, dt in zero_shapes
        ]
        args = [jax.device_put(a, shd) for a in concat_in + concat_zeros]
        jax.block_until_ready(args)
        return args

    def exec_dev(dev_args):
        out_arrs = sharded(*dev_args)
        jax.block_until_ready(out_arrs)
        return out_arrs

    def run(in_maps):
        out_arrs = exec_dev(place(in_maps))
        out_arrs = [np.asarray(a) for a in out_arrs]
        return [
            {
                name: out_arrs[i].reshape(NC_, *zero_shapes[i][0])[c]
                for i, name in enumerate(out_names)
            }
            for c in range(NC_)
        ]

    run.place = place
    run.exec_dev = exec_dev
    run.sharded = sharded
    run.in_names = in_names
    run.out_names = out_names
    run.zero_shapes = zero_shapes
    return run


def _shard_inputs(query, key, value, Wq, bq, Wk, bk, Wv, bv, Wp, bp):
    npdt = getattr(ml_dtypes, _DT_NAME) if _DT_NAME != "float32" else np.float32

    def cvt(x):
        return np.ascontiguousarray(x).astype(npdt)

    qT = [cvt(np.asarray(query[n]).T) for n in range(N)]
    kT = [cvt(np.asarray(key[n]).T) for n in range(N)]
    vT = [cvt(np.asarray(value[n]).T) for n in range(N)]
    in_maps = []
    for n in range(N):
        for hg in range(2):
            sl = slice(hg * EC, (hg + 1) * EC)
            in_maps.append(
                {
                    "qT_in": qT[n],
                    "kT_in": kT[n],
                    "vT_in": vT[n],
                    "wqT": cvt(np.asarray(Wq)[sl, :].T),
                    "wkT": cvt(np.asarray(Wk)[sl, :].T),
                    "wvT": cvt(np.asarray(Wv)[sl, :].T),
                    "wpT": cvt(np.asarray(Wp)[:, sl].T),
                    "bq": np.ascontiguousarray(np.asarray(bq)[sl], dtype=np.float32),
                    "bk": np.ascontiguousarray(np.asarray(bk)[sl], dtype=np.float32),
                    "bv": np.ascontiguousarray(np.asarray(bv)[sl], dtype=np.float32),
                }
            )
    return in_maps


def kernel(query, key, value, Wq, bq, Wk, bk, Wv, bv, Wp, bp):
    nc = _get_nc()
    if "runner" not in _cache:
        _cache["runner"] = _make_runner(nc)
    runner = _cache["runner"]
    in_maps = _shard_inputs(query, key, value, Wq, bq, Wk, bk, Wv, bv, Wp, bp)
    _cache["in_maps"] = in_maps
    results = runner(in_maps)
    bp32 = np.asarray(bp, dtype=np.float32)
    out = np.empty((N, S, E), dtype=np.float32)
    for n in range(N):
        out[n] = (
            results[2 * n]["out"].astype(np.float32)
            + results[2 * n + 1]["out"].astype(np.float32)
            + bp32
        )
    return out


# revision 13
# speedup vs baseline: 1.0011x; 1.0011x over previous
# MultiHeadAttention forward, sharded over 8 NeuronCores.
#
# Problem: N=4, S=T=2048, E=1024, H=16 heads (DH=64), fp32 in/out.
# Sharding: core = n*2 + hg  (n = batch 0..3, hg = head-group 0..1, 8 heads each).
# Each core computes q/k/v projections for its head group, attention, and a
# partial output projection; the host sums the two partials per batch and adds bp.
#
# Device layout (all contractions need the reduced dim on partitions):
#   - host passes query/key/value TRANSPOSED ([E, S]) and the weights
#     pre-transposed, so every matmul operand loads naturally.
#   - qT, kT computed transposed [512, S] (head-dim on partitions); v natural
#     [T, 512]; scores computed transposed [T, S] so softmax normalization can
#     be deferred: exp() is applied PSUM->SBUF on the Scalar engine (scale=1/8
#     fused), y = v_aug.T @ expT accumulates over T with a ones-column in v_aug
#     producing the softmax denominators for free in psum row 64.
#   - K=64 score matmuls are row-packed in pairs of heads (partitions 0-63 /
#     64-127) for PE subarray concurrency.
#
# Schedule (this file's main difference from the straightforward version):
#   - the Scalar engine (exp) is the long pole (~293us of ACTIVATE); the whole
#     kernel is organized to start it ASAP and never let it stall.
#   - DMAs are emitted in deadline order, T-sliced: the 4MB that gates the
#     first exp (wk + kT cols 0:512 + wq + qT cols 0:512) goes first, then v,
#     then everything else.  First exp fires ~25us in instead of ~82us.
#   - attention over all 17 (pair, S-chunk) calls is ONE software-pipelined
#     stream: scores(i) / exp(i) / y(i-1) where i is a global group slot; the
#     y+normalization of a call overlaps the next call's first scores/exp.
#   - all other PE work (v/k/q projections, out-projection) is placed into
#     the stream via a static slot->tasks map, ordered by deadline: v tiles
#     are produced one group ahead of the y that consumes them, k/q units one
#     call ahead, chunk c's out-projection spread across chunk c+1.
#   - the final (pair 3, chunk 3) call is split into two 256-wide half calls
#     so half A's out-projection overlaps half B's attention; two units are
#     held back to bridge the last normalization chain (keeps HAM warm).
#   - softmax denominators: psum row 64 -> partition 0 -> gpsimd
#     partition_broadcast (raw) -> DVE fast-approx reciprocal; head b's
#     normalize writes yt partitions 64:127 directly (cross-base DVE).
#   - outputs staged/stored as bf16 (partials summed on host in fp32).

import numpy as np
import ml_dtypes
from collections import defaultdict

N, S, T, E, H = 4, 2048, 2048, 1024, 16
DH = E // H  # 64
HC = H // 2  # heads per core = 8
EC = HC * DH  # per-core head width = 512
SCALE = float(np.sqrt(DH))  # 8.0
P = 128
KT = E // P  # 8 contraction tiles for projections
NT = T // P  # 16 T-tiles
NC_ = 8  # cores

# matmul/IO dtype on device
_DT_NAME = "bfloat16"

_cache = {}


def _build_nc(debug_dumps=False, reps=1):
    import concourse.bass as bass
    import concourse.mybir as mybir
    import concourse.tile as tile
    from concourse import bacc

    DT = getattr(mybir.dt, _DT_NAME)
    F32 = mybir.dt.float32
    AF = mybir.ActivationFunctionType
    ALU = mybir.AluOpType

    nc = bacc.Bacc("TRN2", target_bir_lowering=False, debug=False, num_devices=NC_)

    qT_in = nc.dram_tensor("qT_in", [E, S], DT, kind="ExternalInput")
    kT_in = nc.dram_tensor("kT_in", [E, T], DT, kind="ExternalInput")
    vT_in = nc.dram_tensor("vT_in", [E, T], DT, kind="ExternalInput")
    wqT = nc.dram_tensor("wqT", [E, EC], DT, kind="ExternalInput")
    wkT = nc.dram_tensor("wkT", [E, EC], DT, kind="ExternalInput")
    wvT = nc.dram_tensor("wvT", [E, EC], DT, kind="ExternalInput")
    wpT = nc.dram_tensor("wpT", [EC, E], DT, kind="ExternalInput")
    bq_d = nc.dram_tensor("bq", [EC], F32, kind="ExternalInput")
    bk_d = nc.dram_tensor("bk", [EC], F32, kind="ExternalInput")
    bv_d = nc.dram_tensor("bv", [EC], F32, kind="ExternalInput")
    out_d = nc.dram_tensor("out", [S, E], DT, kind="ExternalOutput")

    SCH = 512  # S-chunk width (scores rhs free size)
    NCH = S // SCH  # 4 chunks
    TG = 2  # T-tiles per exp group
    NG = NT // TG  # 8 groups per call
    HW = SCH // 2  # width of the two tail half-calls

    inv_scale = 1.0 / SCALE

    with tile.TileContext(nc) as tc:
        with (
            tc.tile_pool(name="const", bufs=1) as const,
            tc.tile_pool(name="wbuf", bufs=1) as wbuf,
            tc.tile_pool(name="kinp", bufs=1) as kinpool,
            tc.tile_pool(name="qin0p", bufs=1) as qin0pool,
            tc.tile_pool(name="bigin", bufs=1) as bigin,
            tc.tile_pool(name="wp", bufs=1) as wppool,
            tc.tile_pool(name="qt", bufs=1) as qtpool,
            tc.tile_pool(name="kt", bufs=1) as ktpool,
            tc.tile_pool(name="vsb", bufs=1) as vpool,
            tc.tile_pool(name="yt", bufs=1) as ytpool,
            tc.tile_pool(name="expb", bufs=8) as expool,
            tc.tile_pool(name="norm", bufs=1) as norm,
            tc.tile_pool(name="ostage", bufs=2) as ostage,
            tc.tile_pool(name="ps_sc", bufs=2, space="PSUM") as ps_sc,
            tc.tile_pool(name="ps_f", bufs=2, space="PSUM") as ps_f,
            tc.tile_pool(name="ps_y", bufs=1, space="PSUM") as ps_y,
        ):
            for _rep in range(reps):
                # ---- exp table preload + PE warm-up burst: get the ~2.7us
                # ACT table-set load and the ~3.4us HAM clock-warm window out
                # of the way while the first DMAs are still in flight ----
                scr = const.tile([1, 16], F32, tag="scr")
                nc.gpsimd.memset(scr[:, :], 0.0)
                nc.scalar.activation(scr[0:1, 8:16], scr[0:1, 0:8], AF.Exp)
                scrw = const.tile([P, SCH], DT, tag="scrw")
                nc.gpsimd.memset(scrw[:, :], 0.0)
                warm_ps = ps_sc.tile([P, TG * SCH], F32, tag="ps", name="warm_ps")

                def warm(n):
                    for _ in range(n):
                        nc.tensor.matmul(
                            warm_ps[:, 0:SCH],
                            lhsT=scrw[:, 0:P],
                            rhs=scrw[:, :],
                            start=True,
                            stop=True,
                        )

                warm(6)

                # ---- constants / biases ----
                bq_sb = const.tile([P, KT // 2], F32, tag="bq")  # [128, 4]
                bk_sb = const.tile([P, KT // 2], F32, tag="bk")
                bvrow = const.tile([P, EC], F32, tag="bvrow")
                bvrep = const.tile([P, EC], F32, tag="bvrep")

                # persistent activation tensors
                qt_t = [
                    qtpool.tile([P, S], DT, tag=f"qt{j}", name=f"qt{j}")
                    for j in range(4)
                ]
                kt_t = [
                    ktpool.tile([P, T], DT, tag=f"kt{j}", name=f"kt{j}")
                    for j in range(4)
                ]
                v_t = [
                    vpool.tile([P, HC * (DH + 1)], DT, tag=f"v{t}", name=f"v{t}")
                    for t in range(NT)
                ]
                yt_t = [
                    ytpool.tile([P, S], DT, tag=f"yt{j}", name=f"yt{j}")
                    for j in range(4)
                ]

                # ones columns of v_aug (col 64 of each 65-wide head block)
                for t in range(NT):
                    vv = v_t[t].rearrange("p (h d) -> p h d", h=HC)
                    nc.gpsimd.memset(vv[:, :, DH : DH + 1], 1.0)

                # ---- inputs/weights: ONE batched 3D DMA per logical block
                # (a dma_start costs ~0.6us of serial Sync-engine issue time,
                # so the exp-gating loads must be few and fat), emitted in
                # deadline order: wk + kT chunk0 + wq + qT chunk0 gate the
                # first exp; v next; everything else streams behind. ----
                # layouts: w*_all[p, (k o)]; kin/vin[p, (cc k t)]
                wk_all = wbuf.tile([P, KT * EC], DT, tag="wk", name="wk_all")
                wq_all = wbuf.tile([P, KT * EC], DT, tag="wq", name="wq_all")
                wv_all = wbuf.tile([P, KT * EC], DT, tag="wv", name="wv_all")
                kin = kinpool.tile([P, KT * T], DT, tag="kin", name="kin_all")
                qin0 = qin0pool.tile([P, KT * SCH], DT, tag="qin0", name="qin0_all")
                vin = bigin.tile([P, KT * T], DT, tag="big", name="vin_all")
                CW = KT * SCH  # columns per (cc) chunk in kin/vin layouts

                def dma_in_chunk(dst_all, src, cc, k0=0, k1=KT):
                    nc.sync.dma_start(
                        dst_all[
                            :, cc * CW + k0 * SCH : cc * CW + k1 * SCH
                        ].rearrange("p (k t) -> p k t", k=k1 - k0),
                        src[k0 * P : k1 * P, cc * SCH : (cc + 1) * SCH].rearrange(
                            "(k p) t -> p k t", p=P
                        ),
                    )

                def dma_w(dst_all, src, k0=0, k1=KT):
                    nc.sync.dma_start(
                        dst_all[:, k0 * EC : k1 * EC].rearrange(
                            "p (k o) -> p k o", k=k1 - k0
                        ),
                        src[k0 * P : k1 * P, :].rearrange("(k p) o -> p k o", p=P),
                    )

                def dma_q0(k0, k1):
                    nc.sync.dma_start(
                        qin0[:, k0 * SCH : k1 * SCH].rearrange(
                            "p (k t) -> p k t", k=k1 - k0
                        ),
                        qT_in[k0 * P : k1 * P, 0:SCH].rearrange(
                            "(k p) t -> p k t", p=P
                        ),
                    )

                # exp-gating loads split in halves so the k/q projection
                # matmuls pipeline behind the DMA instead of waiting for
                # whole-block completion
                dma_w(wk_all, wkT, 0, 4)
                dma_in_chunk(kin, kT_in, 0, 0, 4)
                dma_w(wk_all, wkT, 4, KT)
                dma_in_chunk(kin, kT_in, 0, 4, KT)
                dma_w(wq_all, wqT, 0, 4)
                dma_q0(0, 4)
                dma_w(wq_all, wqT, 4, KT)
                dma_q0(4, KT)
                nc.sync.dma_start(bq_sb[:, :], bq_d.rearrange("(m p) -> p m", p=P))
                nc.sync.dma_start(bk_sb[:, :], bk_d.rearrange("(m p) -> p m", p=P))
                nc.sync.dma_start(bvrow[0:1, :], bv_d[None, :])
                nc.gpsimd.partition_broadcast(bvrep[:, :], bvrow[0:1, :])
                dma_in_chunk(kin, kT_in, 1)
                dma_w(wv_all, wvT)
                dma_in_chunk(vin, vT_in, 0)
                dma_in_chunk(vin, vT_in, 1)
                dma_in_chunk(kin, kT_in, 2)
                dma_in_chunk(kin, kT_in, 3)
                dma_in_chunk(vin, vT_in, 2)
                dma_in_chunk(vin, vT_in, 3)
                # qT chunks 1-3 reuse vin's slot (WAR on the last v-proj read)
                qin_r = bigin.tile([P, KT * (S - SCH)], DT, tag="big", name="qinr")
                for cc in range(3):
                    nc.sync.dma_start(
                        qin_r[:, cc * CW : (cc + 1) * CW].rearrange(
                            "p (k t) -> p k t", k=KT
                        ),
                        qT_in[:, (cc + 1) * SCH : (cc + 2) * SCH].rearrange(
                            "(k p) t -> p k t", p=P
                        ),
                    )
                wp_sb = wppool.tile([P, 4 * E], DT, tag="wp")
                nc.sync.dma_start(
                    wp_sb.rearrange("p (k o) -> p k o", k=4),
                    wpT.rearrange("(k p) o -> p k o", p=P),
                )

                # ---- unit builders (psum from the dedicated 1-bank filler
                # pool so the scores double-buffer rotation stays stable) ----
                def proj_unit(j, w_all, act_slice, bias_sb, dst_slice, split):
                    """act_slice(k) -> rhs AP; returns 1 or 2 emit closures."""
                    state = {}

                    def emit_half(k0, k1, first, last):
                        def emit():
                            if first:
                                state["ps"] = ps_f.tile(
                                    [P, SCH], F32, tag="fps", name="fps"
                                )
                            ps = state["ps"]
                            for k in range(k0, k1):
                                nc.tensor.matmul(
                                    ps[:, :],
                                    lhsT=w_all[:, k * EC + j * P : k * EC + (j + 1) * P],
                                    rhs=act_slice(k),
                                    start=(k == 0),
                                    stop=(k == KT - 1),
                                )
                            if last:
                                nc.vector.tensor_scalar(
                                    dst_slice,
                                    ps[:, :],
                                    bias_sb[:, j : j + 1],
                                    None,
                                    ALU.add,
                                )

                        return emit

                    if split:
                        return [emit_half(0, 4, True, False),
                                emit_half(4, KT, False, True)]
                    return [emit_half(0, KT, True, True)]

                def k_unit(j, cc, split=False):
                    return proj_unit(
                        j, wk_all,
                        lambda k: kin[:, cc * CW + k * SCH : cc * CW + (k + 1) * SCH],
                        bk_sb, kt_t[j][:, cc * SCH : (cc + 1) * SCH], split,
                    )

                def q_unit(j, cc, split=False):
                    if cc == 0:
                        act = lambda k: qin0[:, k * SCH : (k + 1) * SCH]
                    else:
                        act = lambda k: qin_r[
                            :, (cc - 1) * CW + k * SCH : (cc - 1) * CW + (k + 1) * SCH
                        ]
                    return proj_unit(
                        j, wq_all, act, bq_sb,
                        qt_t[j][:, cc * SCH : (cc + 1) * SCH], split,
                    )

                def v_unit(t):
                    def emit():
                        ps = ps_f.tile([P, SCH], F32, tag="fps", name="fps")
                        cc, tl = t // 4, t % 4
                        for k in range(KT):
                            nc.tensor.matmul(
                                ps[:, 0:EC],
                                lhsT=vin[
                                    :,
                                    cc * CW + k * SCH + tl * P :
                                    cc * CW + k * SCH + (tl + 1) * P,
                                ],
                                rhs=wv_all[:, k * EC : (k + 1) * EC],
                                start=(k == 0),
                                stop=(k == KT - 1),
                            )
                        vv = v_t[t].rearrange("p (h d) -> p h d", h=HC)
                        nc.vector.tensor_tensor(
                            vv[:, :, 0:DH],
                            ps[:, 0:EC].rearrange("p (h d) -> p h d", h=HC),
                            bvrep.rearrange("p (h d) -> p h d", h=HC),
                            ALU.add,
                        )

                    return emit

                def out_proj_unit(c, mi, n):
                    def emit():
                        ps = ps_f.tile([P, SCH], F32, tag="fps", name="fps")
                        for kp in range(4):
                            nc.tensor.matmul(
                                ps[:, 0:EC],
                                lhsT=yt_t[kp][
                                    :, c * SCH + mi * P : c * SCH + (mi + 1) * P
                                ],
                                rhs=wp_sb[
                                    :, kp * E + n * EC : kp * E + (n + 1) * EC
                                ],
                                start=(kp == 0),
                                stop=(kp == 3),
                            )
                        ost = ostage.tile([P, EC], DT, tag="ost")
                        nc.vector.tensor_copy(ost[:, :], ps[:, 0:EC])
                        nc.sync.dma_start(
                            out_d[
                                c * SCH + mi * P : c * SCH + (mi + 1) * P,
                                n * EC : (n + 1) * EC,
                            ],
                            ost[:, :],
                        )

                    return emit

                # ---- attention stream pieces ----
                def emit_scores_exp(st, g):
                    j, w, soff = st["j"], st["w"], st["soff"]
                    ps_a = ps_sc.tile([P, TG * SCH], F32, tag="ps", name="ps_a")
                    ps_b = ps_sc.tile([P, TG * SCH], F32, tag="ps", name="ps_b")
                    for i in range(TG):
                        t = g * TG + i
                        nc.tensor.matmul(
                            ps_a[:, i * w : (i + 1) * w],
                            lhsT=kt_t[j][0:DH, t * P : (t + 1) * P],
                            rhs=qt_t[j][0:DH, soff : soff + w],
                            start=True,
                            stop=True,
                        )
                        nc.tensor.matmul(
                            ps_b[:, i * w : (i + 1) * w],
                            lhsT=kt_t[j][DH:P, t * P : (t + 1) * P],
                            rhs=qt_t[j][DH:P, soff : soff + w],
                            start=True,
                            stop=True,
                        )
                    exp_a = expool.tile([P, TG * SCH], DT, tag="exp", name="exp_a")
                    exp_b = expool.tile([P, TG * SCH], DT, tag="exp", name="exp_b")
                    nc.scalar.activation(
                        exp_a[:, 0 : TG * w], ps_a[:, 0 : TG * w], AF.Exp,
                        scale=inv_scale,
                    )
                    nc.scalar.activation(
                        exp_b[:, 0 : TG * w], ps_b[:, 0 : TG * w], AF.Exp,
                        scale=inv_scale,
                    )
                    st["exps"][g] = (exp_a, exp_b)

                def emit_y(st, g):
                    # head b always accumulates at column offset SCH (its own
                    # PSUM bank): matmul start=True clears has_written bits
                    # bank-wide, so the two heads' accumulation groups must
                    # not share a bank even for the half-width tail calls.
                    j, w = st["j"], st["w"]
                    if st["ps_yab"] is None:
                        st["ps_yab"] = ps_y.tile(
                            [P, 2 * SCH], F32, tag="ps_yab", name="ps_yab"
                        )
                    ps_yab = st["ps_yab"]
                    exp_a, exp_b = st["exps"].pop(g)
                    for i in range(TG):
                        t = g * TG + i
                        va = v_t[t][:, (2 * j) * (DH + 1) : (2 * j + 1) * (DH + 1)]
                        vb = v_t[t][
                            :, (2 * j + 1) * (DH + 1) : (2 * j + 2) * (DH + 1)
                        ]
                        nc.tensor.matmul(
                            ps_yab[0 : DH + 1, 0:w],
                            lhsT=va,
                            rhs=exp_a[:, i * w : (i + 1) * w],
                            start=(t == 0),
                            stop=(t == NT - 1),
                        )
                        nc.tensor.matmul(
                            ps_yab[0 : DH + 1, SCH : SCH + w],
                            lhsT=vb,
                            rhs=exp_b[:, i * w : (i + 1) * w],
                            start=(t == 0),
                            stop=(t == NT - 1),
                        )

                def emit_norm(st):
                    # unnormalized y (incl. denominator row 64) copied psum ->
                    # SBUF per head (shortest WAR on the y psum banks for the
                    # next call); raw denominator rows partition-broadcast on
                    # the idle GpSimd engine, then fast-approx DVE reciprocal
                    # (~18 bits, plenty for softmax denoms; base partition
                    # must be 0 for the custom op).  No DMA hops anywhere.
                    j, w, soff = st["j"], st["w"], st["soff"]
                    ps_yab = st["ps_yab"]
                    yun = norm.tile([P, 2 * SCH], F32, tag="yun")
                    den0 = norm.tile([1, 2 * SCH], F32, tag="den0")
                    nc.vector.tensor_copy(
                        den0[0:1, 0:w], ps_yab[DH : DH + 1, 0:w]
                    )
                    nc.vector.tensor_copy(
                        den0[0:1, w : 2 * w], ps_yab[DH : DH + 1, SCH : SCH + w]
                    )
                    nc.vector.tensor_copy(yun[0:DH, 0:w], ps_yab[0:DH, 0:w])
                    nc.vector.tensor_copy(
                        yun[0:DH, w : 2 * w], ps_yab[0:DH, SCH : SCH + w]
                    )
                    recraw = norm.tile([P, 2 * SCH], F32, tag="recraw")
                    nc.gpsimd.partition_broadcast(
                        recraw[0:DH, 0 : 2 * w], den0[0:1, 0 : 2 * w]
                    )
                    recrep = norm.tile([P, 2 * SCH], F32, tag="recrep")
                    nc.vector.reciprocal_approx_fast(
                        recrep[0:DH, 0 : 2 * w], recraw[0:DH, 0 : 2 * w]
                    )
                    nc.vector.tensor_tensor(
                        yt_t[j][0:DH, soff : soff + w],
                        yun[0:DH, 0:w],
                        recrep[0:DH, 0:w],
                        ALU.mult,
                    )
                    # head b writes yt directly at partitions 64:128
                    nc.vector.tensor_tensor(
                        yt_t[j][DH:P, soff : soff + w],
                        yun[0:DH, w : 2 * w],
                        recrep[0:DH, w : 2 * w],
                        ALU.mult,
                    )

                # ---- call list: 15 full calls + 2 half calls for the tail ----
                calls = [
                    {"c": c, "j": j, "soff": c * SCH, "w": SCH,
                     "exps": {}, "ps_yab": None}
                    for c in range(NCH)
                    for j in range(4)
                ]

                # ---- static slot -> filler tasks (deadline-ordered).
                # k/q units appended first within a slot (they gate scores ->
                # ACT), v units after (their consumer, y, has slack). ----
                slot_tasks = defaultdict(list)
                for cc in range(1, 4):  # k(0,0) is emitted pre-stream
                    slot_tasks[2 * cc - 1] += k_unit(0, cc)
                for j in range(1, 4):
                    for cc in range(4):
                        slot_tasks[8 * j + 2 * cc - 2] += k_unit(j, cc)
                    slot_tasks[8 * j - 2] += q_unit(j, 0)
                for c in range(1, 4):
                    for j in range(4):
                        h1, h2 = q_unit(j, c, split=True)
                        slot_tasks[32 * c + 8 * j - 7].append(h1)
                        slot_tasks[32 * c + 8 * j - 6].append(h2)
                for t in range(NT):
                    slot_tasks[t // 2 + 1].append(v_unit(t))
                # out-projection: chunk c spread over chunk c+1; chunk 2's
                # over the chunk-3 calls; chunk 3's handled in the tail.
                for c in range(2):
                    for i, (mi, n) in enumerate(
                        (mi, n) for mi in range(4) for n in range(2)
                    ):
                        slot_tasks[32 * (c + 1) + 2 + i].append(
                            out_proj_unit(c, mi, n)
                        )
                c2_units = [out_proj_unit(2, mi, n) for mi in range(4) for n in range(2)]
                for i, fn in enumerate(c2_units[:5]):
                    slot_tasks[98 + i].append(fn)
                # three chunk-2 units are held back to bridge the final
                # normalization chain (keeps the PE HAM-warm into the tail)
                bridge_units = c2_units[5:]

                tail_units = [out_proj_unit(3, mi, n) for mi in range(4) for n in range(2)]

                # ---- bootstrap: what the first scores need, padded with
                # dependency-free warm-up matmuls so the PE never idles long
                # enough during the DMA-paced ramp for HAM to re-throttle ----
                k00h1, k00h2 = k_unit(0, 0, split=True)
                q00h1, q00h2 = q_unit(0, 0, split=True)
                k00h1()
                warm(4)
                k00h2()
                warm(4)
                q00h1()
                warm(3)
                q00h2()
                warm(2)

                # ---- the software-pipelined stream (y lags 2 slots so a new
                # call's first y never head-of-line blocks the PE queue on
                # the previous call's normalization psum reads) ----
                pending = []
                for ci, st in enumerate(calls):
                    for g in range(NG):
                        slot = 8 * ci + g
                        emit_scores_exp(st, g)
                        if len(pending) == 2:
                            pst, pg = pending.pop(0)
                            emit_y(pst, pg)
                            if pg == NG - 1:
                                emit_norm(pst)
                        pending.append((st, g))
                        for fn in slot_tasks.get(slot, ()):
                            fn()
                # drain: final two y's + normalization, HAM-bridged by the
                # reserved chunk-2 units running during the norm chain
                for pst, pg in pending:
                    emit_y(pst, pg)
                    if pg == NG - 1:
                        for fn in bridge_units:
                            fn()
                        emit_norm(pst)
                for fn in tail_units:
                    fn()

    nc.compile()
    return nc


def _get_nc():
    if "nc" not in _cache:
        _cache["nc"] = _build_nc()
    return _cache["nc"]


def _make_runner(nc):
    """Build a reusable sharded jit callable (mirrors bass2jax.run_bass_via_pjrt)."""
    import jax
    import numpy as np
    import concourse.mybir as mybir
    from concourse import bass2jax
    from concourse.bass2jax import _bass_exec_p, partition_id_tensor
    from jax.sharding import Mesh, PartitionSpec, NamedSharding
    from jax.experimental.shard_map import shard_map

    bass2jax.install_neuronx_cc_hook()

    partition_name = nc.partition_id_tensor.name if nc.partition_id_tensor else None
    in_names, out_names, out_avals, zero_shapes = [], [], [], []
    for alloc in nc.m.functions[0].allocations:
        if not isinstance(alloc, mybir.MemoryLocationSet):
            continue
        name = alloc.memorylocations[0].name
        if alloc.kind == "ExternalInput":
            if name != partition_name:
                in_names.append(name)
        elif alloc.kind == "ExternalOutput":
            out_names.append(name)
            shape = tuple(alloc.tensor_shape)
            dtype = mybir.dt.np(alloc.dtype)
            out_avals.append(jax.core.ShapedArray(shape, dtype))
            zero_shapes.append((shape, dtype))
    n_params = len(in_names)
    n_outs = len(out_avals)
    all_names = list(in_names) + list(out_names)
    if partition_name is not None:
        all_names.append(partition_name)

    def _body(*args):
        operands = list(args)
        if partition_name is not None:
            operands.append(partition_id_tensor())
        outs = _bass_exec_p.bind(
            *operands,
            out_avals=tuple(out_avals),
            in_names=tuple(all_names),
            out_names=tuple(out_names),
            lowering_input_output_aliases=(),
            sim_require_finite=True,
            sim_require_nnan=True,
            nc=nc,
        )
        return tuple(outs)

    devices = jax.devices()[:NC_]
    mesh = Mesh(np.asarray(devices), ("core",))
    in_specs = (PartitionSpec("core"),) * (n_params + n_outs)
    out_specs = (PartitionSpec("core"),) * n_outs
    sharded = jax.jit(
        shard_map(
            _body, mesh=mesh, in_specs=in_specs, out_specs=out_specs, check_rep=False
        ),
        keep_unused=True,
    )

    shd = NamedSharding(mesh, PartitionSpec("core"))

    def place(in_maps):
        """device_put the concatenated inputs + zero out-buffers once."""
        per_core = [[np.asarray(m[name]) for name in in_names] for m in in_maps]
        concat_in = [
            np.concatenate([per_core[c][i] for c in range(NC_)], axis=0)
            for i in range(n_params)
        ]
        concat_zeros = [
            np.zeros((NC_ * sh[0], *sh[1:]), dt) for sh, dt in zero_shapes
        ]
        args = [jax.device_put(a, shd) for a in concat_in + concat_zeros]
        jax.block_until_ready(args)
        return args

    def exec_dev(dev_args):
        out_arrs = sharded(*dev_args)
        jax.block_until_ready(out_arrs)
        return out_arrs

    def run(in_maps):
        out_arrs = exec_dev(place(in_maps))
        out_arrs = [np.asarray(a) for a in out_arrs]
        return [
            {
                name: out_arrs[i].reshape(NC_, *zero_shapes[i][0])[c]
                for i, name in enumerate(out_names)
            }
            for c in range(NC_)
        ]

    run.place = place
    run.exec_dev = exec_dev
    run.sharded = sharded
    run.in_names = in_names
    run.out_names = out_names
    run.zero_shapes = zero_shapes
    return run


def _shard_inputs(query, key, value, Wq, bq, Wk, bk, Wv, bv, Wp, bp):
    npdt = getattr(ml_dtypes, _DT_NAME) if _DT_NAME != "float32" else np.float32

    def cvt(x):
        return np.ascontiguousarray(x).astype(npdt)

    qT = [cvt(np.asarray(query[n]).T) for n in range(N)]
    kT = [cvt(np.asarray(key[n]).T) for n in range(N)]
    vT = [cvt(np.asarray(value[n]).T) for n in range(N)]
    in_maps = []
    for n in range(N):
        for hg in range(2):
            sl = slice(hg * EC, (hg + 1) * EC)
            in_maps.append(
                {
                    "qT_in": qT[n],
                    "kT_in": kT[n],
                    "vT_in": vT[n],
                    "wqT": cvt(np.asarray(Wq)[sl, :].T),
                    "wkT": cvt(np.asarray(Wk)[sl, :].T),
                    "wvT": cvt(np.asarray(Wv)[sl, :].T),
                    "wpT": cvt(np.asarray(Wp)[:, sl].T),
                    "bq": np.ascontiguousarray(np.asarray(bq)[sl], dtype=np.float32),
                    "bk": np.ascontiguousarray(np.asarray(bk)[sl], dtype=np.float32),
                    "bv": np.ascontiguousarray(np.asarray(bv)[sl], dtype=np.float32),
                }
            )
    return in_maps


def kernel(query, key, value, Wq, bq, Wk, bk, Wv, bv, Wp, bp):
    nc = _get_nc()
    if "runner" not in _cache:
        _cache["runner"] = _make_runner(nc)
    runner = _cache["runner"]
    in_maps = _shard_inputs(query, key, value, Wq, bq, Wk, bk, Wv, bv, Wp, bp)
    _cache["in_maps"] = in_maps
    results = runner(in_maps)
    bp32 = np.asarray(bp, dtype=np.float32)
    out = np.empty((N, S, E), dtype=np.float32)
    for n in range(N):
        out[n] = (
            results[2 * n]["out"].astype(np.float32)
            + results[2 * n + 1]["out"].astype(np.float32)
            + bp32
        )
    return out
